# revision 1
# baseline (speedup 1.0000x reference)
"""BoundaryLoss Trainium2 kernel (8 NeuronCores, data-parallel over batch).

Per core (one (21,512,512) image): ce[p] = ln(sum_c exp(x[c,p])) - x[t[p],p],
weighted by w[p] = 1 + 2*boundary[p] and summed; host sums 8 partials / BHW.

Layout: pixels = 32 superblocks x 8192.  A channel chunk packs 4 channels x 32
superblocks onto 128 partitions (p = c_local*32 + pb), so each x load is one
fully-contiguous DRAM region with 16KB descriptors (the tiled-small-descriptor
patterns measured ~100GB/s vs ~315GB/s for contiguous loads).  x is host-cast
to bf16 (compute dtype; halves HBM traffic).  Per chunk: ACT exp -> bf16, DVE
fused (t==c)*x mask in one scalar_tensor_tensor, then a block-ones stationary
matmul reduces channels.  The free dim splits into 4 windows of 2048 mapped to
the 4 PSUM quadrants (tile_position), sums in banks 0-3 / gathered in 4-7 --
PSUM holds both full per-pixel images with zero copies, accumulating across
the 6 channel chunks (start/stop on first/last).  The first chunk's load and
compute are split per 2048-window so the pipeline fills ~25us earlier.

Boundary map: t (u8) loaded flat at offsets 0/+-512 so the vertical 3-tap
any-diff is per-partition elementwise; horizontal 3-tap via free-shifts;
borders zeroed pre-collective; one bf16 AllReduce(add) of the (512,512) map
overlapped with the main loop (emitted mid-loop so its trigger never blocks
x-load issue).  Final: ln(sums)-gath, *w, row-reduce, ones-matmul partition
reduce, scale by 1/BHW, store; host adds the 8 core partials.

DMA engine notes: SWDGE (gpsimd) fans across all 16 SDMA engines and is used
for all bulk traffic; the two HWDGE rings only reach 4 engines here.  Rings
are FIFO, so ordering of issue matters more than queue choice.
"""

import sys

sys.path.insert(0, "/opt/trn_rl_repo")

import numpy as np
import ml_dtypes

import concourse.bass as bass
import concourse.bacc as bacc
import concourse.tile as tile
from concourse import mybir
from concourse import bass_utils

F32 = mybir.dt.float32
BF16 = mybir.dt.bfloat16
U8 = mybir.dt.uint8

C = 21          # channels
H = W = 512
NPIX = H * W    # 262144 pixels per core
FREE = 2048     # free dim of dense pixel layout
NBLK = 128      # pixel blocks (rows of the dense layout)
BPT = 6         # blocks per full tile (6*21 = 126 partitions)
NCORES = 8
NTOT = float(NCORES * NPIX)

Exp = mybir.ActivationFunctionType.Exp
Ln = mybir.ActivationFunctionType.Ln
Copy = mybir.ActivationFunctionType.Copy
op = mybir.AluOpType


def _consts():
    # kxm[p, m] = 1 if p % 32 == m: block-sum over the 4 channels packed per
    # sub-tile (partition p = c_local*32 + block).
    kxm = np.zeros((128, 32), np.float32)
    for p in range(128):
        kxm[p, p % 32] = 1.0
    # cvec[p, s] = absolute channel index of partition p in sub-tile s.
    cvec = np.zeros((128, 7), np.float32)
    for s in range(6):
        cvec[:, s] = 4 * s + np.arange(128) // 32
    cvec[:, 5] = 20.0
    cvec[:, 6] = 2 + np.arange(128) // 32
    return kxm.astype(ml_dtypes.bfloat16), cvec


def build_nc(repeat=1, use_cc=True):
    nc = bacc.Bacc(
        "TRN2",
        target_bir_lowering=False,
        debug=False,
        num_devices=NCORES,
        num_swdge_queues=1,
        dynamic_dma_scratch_size=16384,
    )

    x_d = nc.dram_tensor("x", [C, NPIX], BF16, kind="ExternalInput")
    t_d = nc.dram_tensor("t", [H, W], U8, kind="ExternalInput")
    out_d = nc.dram_tensor("out", [1, 1], F32, kind="ExternalOutput")

    kxm_np, cvec_np = _consts()
    kxm_d = nc.inline_tensor(kxm_np, name="kxm")
    ones_d = nc.inline_tensor(np.ones((128, 1), np.float32), name="ones")
    cvec_d = nc.inline_tensor(cvec_np, name="cvec")

    groups = [list(range(NCORES))]

    with tile.TileContext(nc) as tc:
        with (
            tc.tile_pool(name="singles", bufs=1) as singles,
            tc.tile_pool(name="main", bufs=2) as main,
            tc.tile_pool(name="xpool", bufs=3) as xpool,
            tc.tile_pool(name="bm", bufs=1) as bm,
            tc.tile_pool(name="psum", bufs=1, space="PSUM") as psum,
            tc.tile_pool(name="dram", bufs=1, space="DRAM") as dram,
        ):
            # ---- consts to SBUF ----
            kxm = singles.tile([128, 32], BF16, tag="kxm")
            ones = singles.tile([128, 1], F32, tag="ones")
            nc.sync.dma_start(ones[:], ones_d[:])
            cvec = singles.tile([128, 7], F32, tag="cvec")
            nc.sync.dma_start(kxm[:], kxm_d[:])
            nc.sync.dma_start(cvec[:], cvec_d[:])

            for _rep in range(repeat):
                # ---- phase 2: main loop ----
                # Pixel space = 32 superblocks x 8192.  Sub-tile s packs 4
                # channels x 32 superblocks onto 128 partitions (p = c_local*32
                # + pb); its x data is one fully-contiguous 4MB DRAM region
                # (32KB descriptors).  The free dim splits into 4 windows of
                # 2048; window w accumulates into PSUM quadrant 32w (sums in
                # banks 0-3, gathered in banks 4-7) - all of PSUM, no copies.
                sums = psum.tile([NBLK, FREE], F32, tag="sums")
                gath = psum.tile([NBLK, FREE], F32, tag="gath")
                xv = x_d.ap().rearrange("c (B n) -> c B n", n=8192)  # (21,32,8192)
                tvs = t_d.ap().rearrange("(B r) w -> B (r w)", r=16)  # (32,8192) u8

                tb = singles.tile([128, 8192], U8, tag="tb")
                nc.gpsimd.dma_start(
                    tb[:], tvs[None, :, :].to_broadcast((4, 32, 8192))
                )
                # ---- phase 1: boundary map, dense pixel layout ----
                # tden/tsh/tshm are the flat t image at offsets 0/+512/-512
                # (one image row).  rowdiff at center h lives entirely in
                # partition h//4: rd = (tden != tsh), rdm = (tshm != tden),
                # dv = rd | rdm; then horizontal 3-tap with zeroed borders.
                cc_in = dram.tile([H, W], BF16, tag="cc_in")
                cc_out = dram.tile([H, W], BF16, tag="cc_out")
                tflat = t_d.ap().rearrange("h w -> (h w)")
                tden = bm.tile([128, FREE], U8, tag="bm_tden")
                nc.gpsimd.dma_start(
                    tden[:], tflat.rearrange("(P f) -> P f", P=128)
                )
                tsh = bm.tile([128, FREE], U8, tag="bm_tsh")
                nc.vector.memset(tsh[:], 0)
                nc.gpsimd.dma_start(
                    tsh[0:127, :],
                    tflat[512 : 512 + 127 * 2048].rearrange("(P f) -> P f", P=127),
                )
                nc.gpsimd.dma_start(
                    tsh[127:128, 0:1536], tflat[260608:262144][None, :]
                )
                tshm = bm.tile([128, FREE], U8, tag="bm_tshm")
                nc.vector.memset(tshm[:], 0)
                nc.gpsimd.dma_start(
                    tshm[0:1, 512:2048], tflat[0:1536][None, :]
                )
                nc.gpsimd.dma_start(
                    tshm[1:128, :],
                    tflat[1536 : 1536 + 127 * 2048].rearrange("(P f) -> P f", P=127),
                )
                rd = bm.tile([128, FREE], BF16, tag="bm_rd")
                nc.vector.tensor_tensor(rd[:], tden[:], tsh[:], op.not_equal)
                rdm = bm.tile([128, FREE], BF16, tag="bm_rdm")
                nc.vector.tensor_tensor(rdm[:], tshm[:], tden[:], op.not_equal)
                dv = bm.tile([128, FREE], BF16, tag="bm_dv")
                nc.vector.tensor_tensor(dv[:], rd[:], rdm[:], op.max)
                ca = bm.tile([128, FREE], BF16, tag="bm_ca")
                nc.vector.tensor_tensor(
                    ca[:, 1:2047], dv[:, 0:2046], dv[:, 1:2047], op.max
                )
                nc.vector.tensor_tensor(
                    ca[:, 1:2047], ca[:, 1:2047], dv[:, 2:2048], op.max
                )
                cav = ca[:].rearrange("P (r w) -> P r w", w=W)
                nc.vector.memset(cav[:, :, 0:1], 0.0)
                nc.vector.memset(cav[:, :, 511:512], 0.0)
                nc.vector.memset(ca[0:1, 0:W], 0.0)
                zrow = singles.tile([1, W], BF16, tag="zrow")
                nc.vector.memset(zrow[:], 0.0)
                nc.sync.dma_start(ca[127:128, 3 * W : 4 * W], zrow[:])
                nc.sync.dma_start(
                    cc_in[:].rearrange("(P r) w -> P (r w)", r=4), ca[:]
                )
                # chunks: first sub-tile split 2+2 channels so the first
                # x load (and exp/stt) completes early; then 4x4ch + 1ch tail.
                chunks = [
                    (0, 4, 0), (4, 4, 1), (8, 4, 2),
                    (12, 4, 3), (16, 4, 4), (20, 1, 5),
                ]
                nk = len(chunks)
                for k, (c0, nch, cvc) in enumerate(chunks):
                    pp = 32 * nch

                    x_t = xpool.tile([pp, 8192], BF16, tag="x")
                    dmaeng = nc.scalar if k == 3 else nc.gpsimd
                    if k == 0:
                        # split the first load per 2048-window so compute
                        # starts as soon as the first 0.5MB piece lands
                        for q in range(4):
                            nc.gpsimd.dma_start(
                                x_t[:, 2048 * q : 2048 * (q + 1)],
                                xv[c0 : c0 + nch, :, 2048 * q : 2048 * (q + 1)],
                            )
                    else:
                        dmaeng.dma_start(x_t[:], xv[c0 : c0 + nch, :, :])
                    if k == 3:
                        if use_cc:
                            nc.gpsimd.collective_compute(
                                "AllReduce",
                                op.add,
                                replica_groups=groups,
                                ins=[cc_in.opt()],
                                outs=[cc_out.opt()],
                            )
                        else:
                            cc_out = cc_in
                    npc = 4 if k == 0 else 2  # pieces per chunk
                    fpp = 8192 // npc
                    for h in range(npc):
                        f0 = fpp * h
                        ex = main.tile([pp, fpp], BF16, tag=f"ex{fpp}")
                        nc.scalar.activation(
                            ex[:], x_t[:, f0 : f0 + fpp], Exp
                        )
                        mk = main.tile([pp, fpp], BF16, tag=f"mk{fpp}")
                        nc.vector.scalar_tensor_tensor(
                            mk[:],
                            tb[:pp, f0 : f0 + fpp],
                            cvec[:pp, cvc : cvc + 1],
                            x_t[:, f0 : f0 + fpp],
                            op.is_equal,
                            op.mult,
                        )
                        for wi in range(2048 // (8192 // npc) if False else (fpp // 2048)):
                            w4 = (fpp // 2048) * h + wi  # window index 0..3
                            q0 = 32 * w4
                            for j in range(4):
                                fs = 2048 * wi + 512 * j
                                nc.tensor.matmul(
                                    sums[q0 : q0 + 32, 512 * j : 512 * (j + 1)],
                                    kxm[:pp, :],
                                    ex[:, fs : fs + 512],
                                    start=(k == 0),
                                    stop=(k == nk - 1),
                                    tile_position=(0, q0),
                                    skip_group_check=True,
                                )
                                nc.tensor.matmul(
                                    gath[q0 : q0 + 32, 512 * j : 512 * (j + 1)],
                                    kxm[:pp, :],
                                    mk[:, fs : fs + 512],
                                    start=(k == 0),
                                    stop=(k == nk - 1),
                                    tile_position=(0, q0),
                                    skip_group_check=True,
                                )

                logs = singles.tile([NBLK, FREE], F32, tag="logs")
                nc.scalar.activation(logs[:], sums[:], Ln)
                d = singles.tile([NBLK, FREE], F32, tag="d")
                nc.vector.tensor_tensor(d[:], logs[:], gath[:], op.subtract)
                # ---- phase 3: weight image from reduced boundary map ----
                # psum partition 32w+pb, free n'' <-> pixel pb*8192+w*2048+n''.
                bd = singles.tile([NBLK, FREE], F32, tag="bd")
                ccv = cc_out[:].rearrange("(B r) w -> B (r w)", r=16).rearrange("B (q n) -> B q n", q=4)
                for w4 in range(4):
                    nc.gpsimd.dma_start(
                        bd[32 * w4 : 32 * w4 + 32, :], ccv[:, w4, :]
                    )
                w_img = singles.tile([NBLK, FREE], F32, tag="w_img")
                nc.vector.tensor_scalar(w_img[:], bd[:], 0.0, None, op.is_gt)
                nc.vector.tensor_scalar(w_img[:], w_img[:], 2.0, 1.0, op.mult, op.add)

                # ---- phase 4: final reduction ----
                partials = singles.tile([NBLK, 1], F32, tag="partials")
                nc.vector.tensor_tensor(d[:], d[:], w_img[:], op.mult)
                nc.vector.reduce_sum(partials[:], d[:], axis=mybir.AxisListType.X)
                totp = psum.tile([1, 1], F32, tag="sums")
                nc.tensor.matmul(totp[:], ones[:], partials[:], start=True, stop=True)
                fin = singles.tile([1, 1], F32, tag="fin")
                nc.scalar.activation(fin[:], totp[:], Copy, scale=1.0 / NTOT)

                nc.gpsimd.dma_start(out_d[:], fin[:])

    nc.compile()
    return nc


_NC = None


def _get_nc():
    global _NC
    if _NC is None:
        _NC = build_nc()
    return _NC


def make_in_maps(inputs, targets):
    in_maps = []
    for i in range(NCORES):
        t_i = np.asarray(targets[i])
        in_maps.append(
            {
                "x": np.ascontiguousarray(
                    np.asarray(inputs[i], dtype=np.float32)
                    .reshape(C, NPIX)
                    .astype(ml_dtypes.bfloat16)
                ),
                "t": t_i.astype(np.uint8),
            }
        )
    return in_maps


def run_device(inputs, targets, trace=False):
    nc = _get_nc()
    res = bass_utils.run_bass_kernel_spmd(
        nc,
        make_in_maps(inputs, targets),
        core_ids=list(range(NCORES)),
        trace=trace,
    )
    return res


def kernel(inputs, targets):
    res = run_device(inputs, targets, trace=False)
    # each core returns its local weighted-sum / (B*H*W); the global mean is
    # the sum of the 8 partials (final reduction of the batch shard).
    return np.float32(sum(float(r["out"][0, 0]) for r in res.results))



# revision 11
# speedup vs baseline: 1.2344x; 1.2344x over previous
"""BoundaryLoss Trainium2 kernel v2 (8 NeuronCores, data-parallel over batch).

Per core (one (21,512,512) image): ce[p] = ln(sum_c exp(x[c,p])) - x[t[p],p],
weighted by w[p] = 1 + 2*boundary[p], summed and scaled by 1/(B*H*W); the host
adds the 8 per-core partials.

Layout: channel-serial.  Pixels live in a fixed (128 partitions x 2048) map
(partition p = flat pixels [2048p, 2048p+2048) = image rows 4p..4p+3).  The
host pre-transposes x to [128, 21*2048] fp8_e4m3 so each partition's data is
one contiguous 43KB DRAM run (big descriptors -> full HBM bandwidth; fp8
halves traffic vs bf16; quantization error ~4% rms washes out in the 2.1M-
pixel mean).  Per channel c: ACT exp (fp8 in -> bf16 out), DVE mask
m=(t==c) via tensor_scalar (4x mode; all-bf16 operands), mke=m*exp via
tensor_tensor (2x mode), then identity-stationary matmuls accumulate both
exp and mke images into two [128,2048] f32 PSUM tiles (4 banks each = all 8
banks) across the 21 channels.  scalar_tensor_tensor is NEVER used for bulk
work (it has no DVE fast modes - it was the old kernel's 45us bottleneck).

Boundary map: host sends t3[p] = flat t padded +-512 at [128, 3072] bf16, so
tshm/tden/tsh are three overlapping SBUF views of ONE tensor (no broadcast
loads).  rd/rdm (DVE not_equal, 2x) -> vertical-any; OR + horizontal 3-tap +
u8 convert on GPSIMD (off the critical DVE path); borders zeroed; one EARLY
u8 AllReduce(add) of the 256KB map overlaps the main loop.  bd>0 -> w=1+2b
built on GPSIMD while the loop runs.

Tail is pipelined per PSUM bank (4x512): ln(sums)-ln(gath) (exp/ln roundtrip
keeps every DVE operand 2-byte), *w, ones-matmul partition-reduce into the
freed sums bank row 0, one ACT copy w/ accum_out + 1/N scale, DMA out.

All bulk DMA rides SWDGE (gpsimd) - 16 engines; HWDGE only for tiny consts.
"""

import sys

sys.path.insert(0, "/opt/trn_rl_repo")

import numpy as np
import ml_dtypes

import concourse.bass as bass
import concourse.bacc as bacc
import concourse.tile as tile
from concourse import mybir
from concourse import bass_utils

F32 = mybir.dt.float32
BF16 = mybir.dt.bfloat16
U8 = mybir.dt.uint8
FP8 = mybir.dt.float8e4

C = 21            # channels
H = W = 512
NPIX = H * W      # 262144 pixels per core
P = 128           # SBUF partitions
CW = NPIX // P    # 2048 pixels per partition
XW = C * CW       # 43008 bytes per partition of fp8 x
T3W = CW + 1024   # 3072: t padded with +-512 halo
NCORES = 8
NTOT = float(NCORES * NPIX)
BANK = 512        # PSUM bank width in f32

Exp = mybir.ActivationFunctionType.Exp
Ln = mybir.ActivationFunctionType.Ln
Copy = mybir.ActivationFunctionType.Copy
op = mybir.AluOpType

# x DMA split points (channels): first piece small so exp starts early
X_SPLITS = [(0, 2), (2, 6), (6, 13), (13, 21)]


def build_nc(use_cc=True):
    nc = bacc.Bacc(
        "TRN2",
        target_bir_lowering=False,
        debug=False,
        num_devices=NCORES,
        num_swdge_queues=1,
        dynamic_dma_scratch_size=16384,
    )

    x_d = nc.dram_tensor("x", [P, XW], FP8, kind="ExternalInput")
    t3_d = nc.dram_tensor("t3", [P, T3W], BF16, kind="ExternalInput")
    out_d = nc.dram_tensor("out", [1, 1], F32, kind="ExternalOutput")

    ident_d = nc.inline_tensor(np.eye(P, dtype=ml_dtypes.bfloat16), name="ident")
    ones_d = nc.inline_tensor(np.ones((P, 1), ml_dtypes.bfloat16), name="ones")

    groups = [list(range(NCORES))]

    with tile.TileContext(nc) as tc:
        with (
            tc.tile_pool(name="singles", bufs=1) as singles,
            tc.tile_pool(name="expp", bufs=3) as expp,
            tc.tile_pool(name="mp", bufs=3) as mp,
            tc.tile_pool(name="tailp", bufs=2) as tailp,
            tc.tile_pool(name="psum", bufs=1, space="PSUM") as psum,
            tc.tile_pool(name="dram", bufs=1, space="DRAM") as dram,
        ):
            # tiny consts on HWDGE so the SWDGE queue starts with t3/x
            ident = singles.tile([P, P], BF16, tag="ident")
            ones = singles.tile([P, 1], BF16, tag="ones")
            nc.sync.dma_start(ident[:], ident_d[:])
            nc.sync.dma_start(ones[:], ones_d[:])

            # ---- loads: t3 first (boundary + masks need it), then x ----
            t3 = singles.tile([P, T3W], BF16, tag="t3")
            nc.gpsimd.dma_start(t3[:], t3_d[:])
            x_t = singles.tile([P, XW], FP8, tag="x")
            for a, b in X_SPLITS:
                nc.gpsimd.dma_start(
                    x_t[:, a * CW : b * CW], x_d[:, a * CW : b * CW]
                )

            tshm = t3[:, 0:CW]            # flat t shifted -512 (row above)
            tden = t3[:, 512 : 512 + CW]  # flat t
            tsh = t3[:, 1024 : 1024 + CW]  # flat t shifted +512 (row below)

            # ---- boundary map ----
            # vertical any-diff on DVE (2x mode), the rest on GPSIMD so the
            # DVE queue is free for the mask ops.
            # All on DVE (cheap 2x-mode bf16 adds; masks are 0/1 so `add`
            # doubles as OR), finishing by ~10us so the AllReduce can launch
            # far ahead of when its result is needed.
            rd = singles.tile([P, CW], BF16, tag="rd")
            nc.vector.tensor_tensor(rd[:], tden, tsh, op.not_equal)
            rdm = singles.tile([P, CW], BF16, tag="rdm")
            nc.vector.tensor_tensor(rdm[:], tshm, tden, op.not_equal)
            dv = singles.tile([P, CW], BF16, tag="dv")
            nc.vector.tensor_tensor(dv[:], rd[:], rdm[:], op.add)
            cat = singles.tile([P, CW], BF16, tag="cat")
            nc.vector.tensor_tensor(
                cat[:, 1 : CW - 1], dv[:, 0 : CW - 2], dv[:, 1 : CW - 1], op.add
            )
            ca8 = singles.tile([P, CW], U8, tag="ca8")
            nc.vector.tensor_tensor(
                ca8[:, 1 : CW - 1], cat[:, 1 : CW - 1], dv[:, 2:CW], op.add
            )
            cav = ca8[:].rearrange("P (r w) -> P r w", w=W)
            nc.vector.memset(cav[:, :, 0:1], 0)
            nc.vector.memset(cav[:, :, W - 1 : W], 0)
            nc.vector.memset(ca8[0:1, 0:W], 0)
            # engines can't address a start partition of 127; DMA a zero row
            zrow = singles.tile([1, W], U8, tag="zrow")
            nc.vector.memset(zrow[:], 0)
            nc.sync.dma_start(ca8[P - 1 : P, 3 * W : 4 * W], zrow[:])

            cc_in = dram.tile([P, CW], U8, tag="cc_in")
            cc_out = dram.tile([P, CW], U8, tag="cc_out")
            bd = singles.tile([P, CW], U8, tag="bd")
            w4 = singles.tile([P, CW], BF16, tag="w4")
            nc.gpsimd.dma_start(cc_in[:], ca8[:])
            if use_cc:
                nc.gpsimd.collective_compute(
                    "AllReduce",
                    op.add,
                    replica_groups=groups,
                    ins=[cc_in.opt()],
                    outs=[cc_out.opt()],
                )
            else:
                cc_out = cc_in
            nc.gpsimd.dma_start(bd[:], cc_out[:])

            # ---- main loop: 21 channels ----
            sums = psum.tile([P, CW], F32, tag="sums")
            gath = psum.tile([P, CW], F32, tag="gath")
            for c in range(C):
                if c == 17:
                    # weights w = 1 + 2*(bd > 0), overlapping the loop
                    nc.vector.tensor_scalar(
                        w4[:], bd[:], 0.0, 2.0, op.is_gt, op.mult
                    )
                    nc.vector.tensor_scalar(w4[:], w4[:], 1.0, None, op.add)
                ex = expp.tile([P, CW], BF16, tag="ex")
                nc.scalar.activation(ex[:], x_t[:, c * CW : (c + 1) * CW], Exp)
                m = mp.tile([P, CW], BF16, tag="m")
                nc.vector.tensor_scalar(m[:], tden, float(c), None, op.is_equal)
                mke = mp.tile([P, CW], BF16, tag="mke")
                nc.vector.tensor_tensor(mke[:], m[:], ex[:], op.mult)
                for j in range(4):
                    s = slice(j * BANK, (j + 1) * BANK)
                    nc.tensor.matmul(
                        sums[:, s],
                        ident[:],
                        ex[:, s],
                        start=(c == 0),
                        stop=(c == C - 1),
                        skip_group_check=True,
                    )
                for j in range(4):
                    s = slice(j * BANK, (j + 1) * BANK)
                    nc.tensor.matmul(
                        gath[:, s],
                        ident[:],
                        mke[:, s],
                        start=(c == 0),
                        stop=(c == C - 1),
                        skip_group_check=True,
                    )

            # ---- tail, pipelined per PSUM bank ----
            for j in range(4):
                s = slice(j * BANK, (j + 1) * BANK)
                logs = tailp.tile([P, BANK], BF16, tag="logs")
                nc.scalar.activation(logs[:], sums[:, s], Ln)
                logg = tailp.tile([P, BANK], BF16, tag="logg")
                nc.scalar.activation(logg[:], gath[:, s], Ln)
                ce = tailp.tile([P, BANK], BF16, tag="ce")
                nc.vector.tensor_tensor(ce[:], logs[:], logg[:], op.subtract)
                wce = tailp.tile([P, BANK], BF16, tag="wce")
                nc.vector.tensor_tensor(wce[:], ce[:], w4[:, s], op.mult)
                # partition-reduce into row 0 of the (now-consumed) sums bank
                nc.tensor.matmul(
                    sums[0:1, s],
                    ones[:],
                    wce[:],
                    start=True,
                    stop=True,
                    skip_group_check=True,
                )

            scr = singles.tile([1, CW], BF16, tag="scr")
            fin = singles.tile([1, 1], F32, tag="fin")
            nc.scalar.activation(
                scr[:], sums[0:1, :], Copy, scale=1.0 / NTOT, accum_out=fin[:]
            )
            nc.gpsimd.dma_start(out_d[:], fin[:])

    nc.compile()
    return nc


_NC = None


def _get_nc():
    global _NC
    if _NC is None:
        _NC = build_nc()
    return _NC


def make_in_maps(inputs, targets):
    e4 = ml_dtypes.float8_e4m3
    in_maps = []
    for i in range(NCORES):
        x = np.asarray(inputs[i], dtype=np.float32).reshape(C, P, CW)
        x8 = np.ascontiguousarray(x.transpose(1, 0, 2)).astype(e4).reshape(P, XW)
        t = np.asarray(targets[i]).astype(np.uint8).reshape(-1)
        tp = np.zeros(NPIX + 1024, np.uint8)
        tp[512 : 512 + NPIX] = t
        t3 = np.lib.stride_tricks.as_strided(tp, (P, T3W), (CW, 1))
        t3 = np.ascontiguousarray(t3).astype(ml_dtypes.bfloat16)
        in_maps.append({"x": x8, "t3": t3})
    return in_maps


def run_device(inputs, targets, trace=False):
    nc = _get_nc()
    res = bass_utils.run_bass_kernel_spmd(
        nc,
        make_in_maps(inputs, targets),
        core_ids=list(range(NCORES)),
        trace=trace,
    )
    return res


def kernel(inputs, targets):
    res = run_device(inputs, targets, trace=False)
    # each core returns its local weighted-sum / (B*H*W); the global mean is
    # the sum of the 8 partials (final reduction of the batch shard).
    return np.float32(sum(float(r["out"][0, 0]) for r in res.results))
